# revision 10
# baseline (speedup 1.0000x reference)
"""Trainium2 Bass kernel for the hard-negative-mining set loss (v5).

Per-core structure (rows data-parallel, 1024 local anchors):
  * phase B: per-class first/second local occurrence via MAX8+FI8 on a
    bf16 one-hot eqB (runs during input DMA)
  * phase A: exp(x-10) with accumulated rowsum; PE transpose with
    rhs=diag(-1/rsum) folds the hardness scale into the transpose;
    same-class mask applied by one fused stt reading PSUM; MAX8+FI8
    per class; batched wide encodes
  * one 16KB AllGather; wide strided-AP combine; swapped-operand bf16
    resolution matmuls (hi*128 + lo exact split) -> per-anchor pos/neg
  * batched bf16 row gathers (one indirect DMA per row tile) from a
    host-cast bf16 copy of x; phase C sums in bf16 with fused
    tensor_tensor_reduce target-logit extraction
"""

import numpy as np
import ml_dtypes

import concourse.bass as bass
import concourse.bacc as bacc
import concourse.tile as tile
from concourse import mybir
from concourse.bass_utils import run_bass_kernel_spmd

B, C = 8192, 1024
NCORES = 8
BL = B // NCORES      # 1024 local rows per core
NT = BL // 128        # 8 row tiles
CT = C // 128         # 8 class tiles
BIGI = 16384.0        # index encoding base: enc = BIGI - global_row_idx
SHIFT_A = 10.0        # softmax shift (x ~ N(0,1): rowmax << SHIFT_A)
SHIFT_C = 14.0        # summed-logits shift (3 logits per entry)
F32 = mybir.dt.float32
F16 = mybir.dt.float16
BF16 = mybir.dt.bfloat16
I32 = mybir.dt.int32
U32 = mybir.dt.uint32
AX = mybir.AxisListType
OP = mybir.AluOpType
AF = mybir.ActivationFunctionType

# dtype for the mined hardness path (F32 = exact vs reference argmin;
# BF16 = 2x DVE rate, tiny tie-flip risk)
MINE_DT = F32


def ap3(t_, dims, off=0):
    """Strided free-dim view of a tile: dims = [[stride, size], ...]."""
    return bass.AP(tensor=t_.tensor, offset=t_.offset + off,
                   ap=[t_.ap[0]] + dims)


def build_nc():
    nc = bacc.Bacc("TRN2", target_bir_lowering=False, debug=False,
                   num_devices=NCORES)

    xbf_d = nc.dram_tensor("xbf", [B, C], BF16, kind="ExternalInput")
    xloc_d = nc.dram_tensor("xloc", [BL, C], F32, kind="ExternalInput")
    tgtb_d = nc.dram_tensor("tgtb", [128, BL], F16, kind="ExternalInput")
    cidb_d = nc.dram_tensor("cidb", [128, C], F32, kind="ExternalInput")
    ident_d = nc.dram_tensor("ident", [128, 128], F32, kind="ExternalInput")
    cidcol_d = nc.dram_tensor("cidcol", [128, CT], F16, kind="ExternalInput")
    tcols_d = nc.dram_tensor("tcols", [128, NT], F32, kind="ExternalInput")
    gidxcol_d = nc.dram_tensor("gidxcol", [128, NT], F32, kind="ExternalInput")
    negjb_d = nc.dram_tensor("negjb", [128, BL], F32, kind="ExternalInput")
    bigoff_d = nc.dram_tensor("bigoff", [128, 1], F32, kind="ExternalInput")
    out_d = nc.dram_tensor("partial", [1, 1], F32, kind="ExternalOutput")

    # collective payload, per class ct*128+p:
    #   col ct        = vmax   (max over local rows of -hardness)
    #   col CT+ct     = negenc (BIGI - global row of local argmax, 0-masked)
    #   col 2CT+2ct+k = posenc (k=0 first, k=1 second local row enc)
    cc_in = nc.dram_tensor("cc_in", [128, 4 * CT], F32)
    cc_out = nc.dram_tensor("cc_out", [NCORES, 128, 4 * CT], F32)

    with tile.TileContext(nc) as tc:
        with (
            tc.tile_pool(name="persist", bufs=1) as pp,
            tc.tile_pool(name="scratch", bufs=2) as sp,
            tc.tile_pool(name="small", bufs=4) as smp,
            tc.tile_pool(name="gather", bufs=4) as gp,
            tc.tile_pool(name="psA", bufs=1, space="PSUM") as psa,
            tc.tile_pool(name="psR", bufs=1, space="PSUM") as psr,
        ):
            # ---------- tiny inputs + on-chip constants ----------
            tgtb = pp.tile([128, BL], F16, tag="tgtb")
            nc.sync.dma_start(out=tgtb, in_=tgtb_d.ap())
            cidcol = pp.tile([128, CT], F16, tag="cidcol")
            nc.sync.dma_start(out=cidcol, in_=cidcol_d.ap())
            tcols = pp.tile([128, NT], F32, tag="tcols")
            nc.sync.dma_start(out=tcols, in_=tcols_d.ap())
            gidxcol = pp.tile([128, NT], F32, tag="gidxcol")
            nc.sync.dma_start(out=gidxcol, in_=gidxcol_d.ap())
            negjb = pp.tile([128, BL], F32, tag="negjb")
            nc.sync.dma_start(out=negjb, in_=negjb_d.ap())
            bigoff = pp.tile([128, 1], F32, tag="bigoff")
            nc.sync.dma_start(out=bigoff, in_=bigoff_d.ap())

            cidbf = pp.tile([128, C], F32, tag="cidbf")
            nc.sync.dma_start(out=cidbf, in_=cidb_d.ap())
            identf = pp.tile([128, 128], F32, tag="identf")
            nc.sync.dma_start(out=identf, in_=ident_d.ap())
            ones = pp.tile([128, 1], F32, tag="ones")
            nc.vector.memset(ones, 1.0)
            shA = pp.tile([128, 1], F32, tag="shA")
            nc.vector.memset(shA, -SHIFT_A)
            shC = pp.tile([128, 1], F32, tag="shC")
            nc.vector.memset(shC, -SHIFT_C)

            # ---------- xloc loads (gate phase A) ----------
            xloc = []
            for t in range(NT):
                xt = pp.tile([128, C], F32, tag=f"xloc{t}")
                nc.sync.dma_start(out=xt, in_=xloc_d.ap()[t * 128:(t + 1) * 128, :])
                xloc.append(xt)

            ccall = pp.tile([128, 4 * CT], F32, tag="ccall")

            # ---------- phase B: first/second local row per class ----------
            # eqB on GpSimd (frees Vector); enc mined directly as
            # eqB * (BIGI - global row) -> top2 via MAX8, no FI8 needed
            eqB = []
            for ct in range(CT):
                eb = pp.tile([128, BL], BF16, tag=f"eqB{ct}")
                nc.vector.tensor_tensor(
                    out=eb, in0=tgtb,
                    in1=cidcol[:, ct:ct + 1].to_broadcast([128, BL]),
                    op=OP.is_equal)
                eqB.append(eb)
            sbv = pp.tile([128, 8 * CT], F32, tag="sbv")
            for ct in range(CT):
                enb = sp.tile([128, BL], F32, tag="enb")
                nc.gpsimd.tensor_tensor(out=enb, in0=eqB[ct], in1=negjb,
                                        op=OP.mult)
                nc.vector.max(out=sbv[:, ct * 8:(ct + 1) * 8], in_=enb)
            nc.vector.tensor_copy(out=ap3(ccall, [[2, CT], [1, 2]], off=2 * CT),
                                  in_=ap3(sbv, [[8, CT], [1, 2]]))

            # ---------- phase A: hardest-negative mining ----------
            dumps = []
            diags = []
            for t in range(NT):
                dump = pp.tile([128, C], MINE_DT, tag=f"dump{t}")
                rsum = smp.tile([128, 1], F32, tag="rsum")
                nc.scalar.activation(out=dump, in_=xloc[t], func=AF.Exp,
                                     bias=shA, scale=1.0, accum_out=rsum)
                rr = smp.tile([128, 1], F32, tag="rr")
                nc.vector.reciprocal(out=rr, in_=rsum)
                dg = pp.tile([128, 128], MINE_DT, tag=f"diag{t}")
                nc.vector.tensor_scalar(out=dg, in0=identf, scalar1=rr,
                                        scalar2=-1.0, op0=OP.mult, op1=OP.mult)
                dumps.append(dump)
                diags.append(dg)

            sav = pp.tile([128, 8 * CT], MINE_DT, tag="sav")
            sai = pp.tile([128, 8 * CT], U32, tag="sai")
            for g in range(4):
                psts = []
                for ci in range(2):
                    pst = psa.tile([128, C], F32, tag=f"pst{ci}")
                    psts.append(pst)
                    ct = g * 2 + ci
                    for t in range(NT):
                        nc.tensor.matmul(
                            pst[:, t * 128:(t + 1) * 128],
                            lhsT=dumps[t][:, ct * 128:(ct + 1) * 128],
                            rhs=diags[t], start=True, stop=True)
                for ci in range(2):
                    ct = g * 2 + ci
                    masked = sp.tile([128, C], MINE_DT, tag="masked")
                    nc.vector.scalar_tensor_tensor(
                        out=masked, in0=eqB[ct], scalar=-512.0,
                        in1=psts[ci], op0=OP.mult, op1=OP.add)
                    nc.vector.max(out=sav[:, ct * 8:(ct + 1) * 8], in_=masked)
                    nc.vector.max_index(out=sai[:, ct * 8:(ct + 1) * 8],
                                        in_max=sav[:, ct * 8:(ct + 1) * 8],
                                        in_values=masked)
            # wide encode
            nc.vector.tensor_copy(out=ccall[:, 0:CT], in_=ap3(sav, [[8, CT]]))
            aidx = smp.tile([128, CT], F32, tag="aidx")
            nc.vector.tensor_copy(out=aidx, in_=ap3(sai, [[8, CT]]))
            nc.vector.tensor_scalar(out=ccall[:, CT:2 * CT], in0=aidx,
                                    scalar1=bigoff, scalar2=-1.0,
                                    op0=OP.subtract, op1=OP.mult)

            nc.sync.dma_start(out=cc_in.ap(), in_=ccall)

            # ---------- AllGather ----------
            nc.gpsimd.collective_compute(
                "AllGather", OP.bypass,
                replica_groups=[list(range(NCORES))],
                ins=[cc_in.ap().opt()], outs=[cc_out.ap().opt()])

            # bubble work: eqm onehots + bf16 xloc for phase C
            eqm = []
            xlb = []
            for t in range(NT):
                em = pp.tile([128, C], BF16, tag=f"eqm{t}")
                nc.vector.tensor_scalar(out=em, in0=cidbf,
                                        scalar1=tcols[:, t:t + 1],
                                        scalar2=None, op0=OP.is_equal)
                eqm.append(em)
                xb = pp.tile([128, C], BF16, tag=f"xlb{t}")
                nc.scalar.copy(out=xb, in_=xloc[t])
                xlb.append(xb)

            # g8[p, core, qct] <- cc_out[core, p, qct]  (qct innermost, contig)
            NC8 = NCORES
            g8 = pp.tile([128, NC8 * 4 * CT], F32, tag="g8")
            gsrc = bass.AP(tensor=cc_out.ap().tensor, offset=0,
                           ap=[[4 * CT, 128], [128 * 4 * CT, NC8], [1, 4 * CT]])
            nc.scalar.dma_start(out=g8, in_=gsrc)

            # ---------- wide combine ----------
            W = 4 * CT
            negdims = [[1, CT], [W, NC8]]      # (ct, core), core innermost
            idx3 = smp.tile([128, 3 * CT], F32, tag="idx3")
            # neg: winner core by vmax, tie -> max enc
            gv = smp.tile([128, CT], F32, tag="gv")
            nc.vector.tensor_reduce(out=gv, in_=ap3(g8, negdims),
                                    axis=AX.X, op=OP.max)
            mmt = smp.tile([128, CT * NC8], F32, tag="mmt")
            mmdims = [[NC8, CT], [1, NC8]]
            nc.vector.tensor_tensor(out=ap3(mmt, mmdims),
                                    in0=ap3(g8, negdims),
                                    in1=ap3(gv, [[1, CT], [0, NC8]]),
                                    op=OP.is_ge)
            cand = smp.tile([128, CT * NC8], F32, tag="cand")
            nc.vector.tensor_tensor(out=ap3(cand, mmdims),
                                    in0=ap3(mmt, mmdims),
                                    in1=ap3(g8, negdims, off=CT),
                                    op=OP.mult)
            genc = smp.tile([128, CT], F32, tag="genc")
            nc.vector.tensor_reduce(out=genc, in_=ap3(cand, mmdims),
                                    axis=AX.X, op=OP.max)
            nc.vector.tensor_scalar(out=idx3[:, CT:2 * CT], in0=genc,
                                    scalar1=-1.0, scalar2=BIGI,
                                    op0=OP.mult, op1=OP.add)

            # pos: top2 enc of 16 candidates (2 local x 8 cores) per class
            g1e = smp.tile([128, CT], F32, tag="g1e")
            g2e = smp.tile([128, CT], F32, tag="g2e")
            for ct in range(CT):
                catsl = ap3(g8, [[W, NC8], [1, 2]], off=2 * CT + 2 * ct)
                top8 = smp.tile([128, 8], F32, tag="topg")
                nc.vector.max(out=top8, in_=catsl)
                nc.vector.tensor_copy(out=g1e[:, ct:ct + 1], in_=top8[:, 0:1])
                nc.vector.tensor_copy(out=g2e[:, ct:ct + 1], in_=top8[:, 1:2])
            nc.vector.tensor_scalar(out=idx3[:, 0:CT], in0=g1e,
                                    scalar1=-1.0, scalar2=BIGI,
                                    op0=OP.mult, op1=OP.add)
            g2i = smp.tile([128, CT], F32, tag="g2i")
            nc.vector.tensor_scalar(out=g2i, in0=g2e, scalar1=-1.0,
                                    scalar2=BIGI, op0=OP.mult, op1=OP.add)
            m2 = smp.tile([128, CT], F32, tag="m2")
            nc.vector.tensor_scalar(out=m2, in0=g2i, scalar1=BIGI,
                                    scalar2=None, op0=OP.is_lt)
            nc.vector.tensor_tensor(out=idx3[:, 2 * CT:3 * CT], in0=g2i,
                                    in1=m2, op=OP.mult)

            # ---------- bf16 hi/lo table ----------
            # idx = hi*128 + lo; tab cols ct*6+q hold hi*128, ct*6+3+q hold lo
            hsc = smp.tile([128, 3 * CT], F32, tag="hsc")
            nc.vector.tensor_scalar(out=hsc, in0=idx3, scalar1=1.0 / 128.0,
                                    scalar2=None, op0=OP.mult)
            hii = smp.tile([128, 3 * CT], I32, tag="hii")
            nc.vector.tensor_copy(out=hii, in_=hsc)  # trunc toward zero
            hif = smp.tile([128, 3 * CT], F32, tag="hif")
            nc.vector.tensor_copy(out=hif, in_=hii)
            hi128 = smp.tile([128, 3 * CT], F32, tag="hi128")
            nc.vector.tensor_scalar(out=hi128, in0=hif, scalar1=128.0,
                                    scalar2=None, op0=OP.mult)
            lof = smp.tile([128, 3 * CT], F32, tag="lof")
            nc.vector.tensor_tensor(out=lof, in0=idx3, in1=hi128, op=OP.subtract)
            tab = pp.tile([128, 6 * CT], BF16, tag="tab")
            nc.vector.tensor_copy(out=ap3(tab, [[1, 3], [6, CT]]),
                                  in_=ap3(hi128, [[CT, 3], [1, CT]]))
            nc.vector.tensor_copy(out=ap3(tab, [[1, 3], [6, CT]], off=3),
                                  in_=ap3(lof, [[CT, 3], [1, CT]]))

            # ---------- resolution ----------
            # ps6[q, j] = hi128_q[class(j)] (rows 0-2) / lo_q[class(j)] (3-5)
            ps6 = psr.tile([6, BL], F32, tag="ps6")
            for h in range(2):
                cols = slice(h * 512, (h + 1) * 512)
                for ct in range(CT):
                    nc.tensor.matmul(ps6[:, cols],
                                     lhsT=tab[:, ct * 6:(ct + 1) * 6],
                                     rhs=eqB[ct][:, cols],
                                     start=(ct == 0), stop=(ct == CT - 1))
            sb6 = pp.tile([6, BL], F32, tag="sb6")
            nc.scalar.copy(out=sb6, in_=ps6)

            # transpose per tile -> offp6[anchor, t*6 + {hi_q | lo_q}]
            offp6 = psr.tile([128, 6 * NT], F32, tag="offp6")
            for t in range(NT):
                nc.tensor.matmul(offp6[:, t * 6:(t + 1) * 6],
                                 lhsT=sb6[:, t * 128:(t + 1) * 128],
                                 rhs=identf[0:6, 0:6], start=True, stop=True)
            offs6 = smp.tile([128, 6 * NT], F32, tag="offs6")
            nc.scalar.copy(out=offs6, in_=offp6)
            # idxw[:, t*3+q] = hi*128 + lo
            idxw = smp.tile([128, 3 * NT], F32, tag="idxw")
            tq = [[3, NT], [1, 3]]
            nc.vector.tensor_tensor(out=ap3(idxw, tq),
                                    in0=ap3(offs6, [[6, NT], [1, 3]]),
                                    in1=ap3(offs6, [[6, NT], [1, 3]], off=3),
                                    op=OP.add)
            # pos = (g1 == self) ? p2z : g1   (wide, strided; q: 0=g1 1=neg 2=p2)
            m1 = smp.tile([128, NT], F32, tag="m1")
            nc.vector.tensor_tensor(out=m1, in0=ap3(idxw, [[3, NT]]),
                                    in1=gidxcol, op=OP.is_equal)
            dsel = smp.tile([128, NT], F32, tag="dsel")
            nc.vector.tensor_tensor(out=dsel, in0=ap3(idxw, [[3, NT]], off=2),
                                    in1=ap3(idxw, [[3, NT]]), op=OP.subtract)
            nc.vector.tensor_tensor(out=dsel, in0=dsel, in1=m1, op=OP.mult)
            posf = smp.tile([128, NT], F32, tag="posf")
            nc.vector.tensor_tensor(out=posf, in0=ap3(idxw, [[3, NT]]),
                                    in1=dsel, op=OP.add)
            offi = pp.tile([128, 2 * NT], I32, tag="offi")
            nc.vector.tensor_copy(out=ap3(offi, [[2, NT]]), in_=posf)
            nc.vector.tensor_copy(out=ap3(offi, [[2, NT]], off=1),
                                  in_=ap3(idxw, [[3, NT]], off=1))

            # ---------- gathers + phase C ----------
            lnstage = pp.tile([128, NT], F32, tag="lnstage")
            tvstage = pp.tile([128, NT], F32, tag="tvstage")
            exps3 = []
            for t in range(NT):
                gpair = gp.tile([128, 2 * C], BF16, tag="gpair")
                nc.gpsimd.indirect_dma_start(
                    out=gpair[:, 0:C], out_offset=None, in_=xbf_d.ap(),
                    in_offset=bass.IndirectOffsetOnAxis(
                        ap=offi[:, 2 * t:2 * t + 1], axis=0))
                nc.gpsimd.indirect_dma_start(
                    out=gpair[:, C:2 * C], out_offset=None, in_=xbf_d.ap(),
                    in_offset=bass.IndirectOffsetOnAxis(
                        ap=offi[:, 2 * t + 1:2 * t + 2], axis=0))
                sumpn = sp.tile([128, C], BF16, tag="sumpn")
                nc.vector.tensor_tensor(out=sumpn, in0=gpair[:, 0:C],
                                        in1=gpair[:, C:2 * C], op=OP.add)
                sum3 = sp.tile([128, C], BF16, tag="sum3")
                nc.vector.tensor_tensor(out=sum3, in0=sumpn, in1=xlb[t],
                                        op=OP.add)
                dumpc = sp.tile([128, C], BF16, tag="dumpc")
                e = nc.scalar.activation(out=dumpc, in_=sum3, func=AF.Exp,
                                         bias=shC, scale=1.0,
                                         accum_out=lnstage[:, t:t + 1])
                exps3.append(e)
                junk = sp.tile([128, C], BF16, tag="junk")
                nc.vector.tensor_tensor(out=junk, in0=sum3, in1=eqm[t],
                                        op=OP.mult)
                nc.vector.tensor_reduce(out=tvstage[:, t:t + 1], in_=junk,
                                        axis=AX.X, op=OP.add)

            lns = pp.tile([128, NT], F32, tag="lns")
            ln = nc.scalar.activation(out=lns, in_=lnstage, func=AF.Ln)
            tile.add_dep_helper(ln.ins, exps3[-1].ins, sync=False)
            li8 = smp.tile([128, NT], F32, tag="li8")
            nc.vector.tensor_scalar(out=li8, in0=lns, scalar1=SHIFT_C,
                                    scalar2=None, op0=OP.add)
            nc.vector.tensor_tensor(out=li8, in0=li8, in1=tvstage,
                                    op=OP.subtract)
            acc = smp.tile([128, 1], F32, tag="acc")
            nc.vector.tensor_reduce(out=acc, in_=li8, axis=AX.X, op=OP.add)

            pss = psr.tile([1, 1], F32, tag="psout")
            nc.tensor.matmul(pss, lhsT=acc, rhs=ones, start=True, stop=True)
            outt = smp.tile([1, 1], F32, tag="outt")
            nc.vector.tensor_copy(out=outt, in_=pss)
            nc.sync.dma_start(out=out_d.ap(), in_=outt)

    nc.compile()
    return nc


_NC_CACHE = {}


def get_nc():
    if "nc" not in _NC_CACHE:
        _NC_CACHE["nc"] = build_nc()
    return _NC_CACHE["nc"]


def make_in_maps(x, target):
    x = np.ascontiguousarray(np.asarray(x, dtype=np.float32))
    tgt = np.asarray(target).astype(np.int64)
    assert x.shape == (B, C) and tgt.shape == (B,)

    xbf = np.ascontiguousarray(x.astype(ml_dtypes.bfloat16))
    cid = np.arange(C, dtype=np.float32)
    cidb_full = np.ascontiguousarray(np.broadcast_to(cid, (128, C)))
    ident_full = np.eye(128, dtype=np.float32)
    cidcol = np.ascontiguousarray(cid.reshape(CT, 128).T.astype(np.float16))

    in_maps = []
    for k in range(NCORES):
        rows = slice(k * BL, (k + 1) * BL)
        tl = tgt[rows].astype(np.float32)
        gi = (k * BL + np.arange(BL)).astype(np.float32)
        in_maps.append({
            "xbf": xbf,
            "xloc": np.ascontiguousarray(x[rows]),
            "cidb": cidb_full,
            "ident": ident_full,
            "tgtb": np.ascontiguousarray(
                np.broadcast_to(tl.astype(np.float16), (128, BL))),
            "cidcol": cidcol,
            "tcols": np.ascontiguousarray(tl.reshape(NT, 128).T),
            "gidxcol": np.ascontiguousarray(gi.reshape(NT, 128).T),
            "negjb": np.ascontiguousarray(np.broadcast_to(BIGI - gi, (128, BL))),
            "bigoff": np.full((128, 1), BIGI - k * BL, dtype=np.float32),
        })
    return in_maps


def kernel(x, target):
    nc = get_nc()
    in_maps = make_in_maps(x, target)
    res = run_bass_kernel_spmd(nc, in_maps, core_ids=list(range(NCORES)))
    total = sum(float(res.results[k]["partial"][0, 0]) for k in range(NCORES))
    return np.float32(total / B)


# revision 11
# speedup vs baseline: 1.2365x; 1.2365x over previous
"""Trainium2 Bass kernel for the hard-negative-mining set loss (v5).

Per-core structure (rows data-parallel, 1024 local anchors):
  * phase B: per-class first/second local occurrence via MAX8+FI8 on a
    bf16 one-hot eqB (runs during input DMA)
  * phase A: exp(x-10) with accumulated rowsum; PE transpose with
    rhs=diag(-1/rsum) folds the hardness scale into the transpose;
    same-class mask applied by one fused stt reading PSUM; MAX8+FI8
    per class; batched wide encodes
  * one 16KB AllGather; wide strided-AP combine; swapped-operand bf16
    resolution matmuls (hi*128 + lo exact split) -> per-anchor pos/neg
  * batched bf16 row gathers (one indirect DMA per row tile) from a
    host-cast bf16 copy of x; phase C sums in bf16 with fused
    tensor_tensor_reduce target-logit extraction
"""

import numpy as np
import ml_dtypes

import concourse.bass as bass
import concourse.bacc as bacc
import concourse.tile as tile
from concourse import mybir
from concourse.bass_utils import run_bass_kernel_spmd

B, C = 8192, 1024
NCORES = 8
BL = B // NCORES      # 1024 local rows per core
NT = BL // 128        # 8 row tiles
CT = C // 128         # 8 class tiles
BIGI = 16384.0        # index encoding base: enc = BIGI - global_row_idx
SHIFT_A = 10.0        # softmax shift (x ~ N(0,1): rowmax << SHIFT_A)
SHIFT_C = 14.0        # summed-logits shift (3 logits per entry)
F32 = mybir.dt.float32
F16 = mybir.dt.float16
BF16 = mybir.dt.bfloat16
I32 = mybir.dt.int32
U32 = mybir.dt.uint32
AX = mybir.AxisListType
OP = mybir.AluOpType
AF = mybir.ActivationFunctionType

# dtype for the mined hardness path (F32 = exact vs reference argmin;
# BF16 = 2x DVE rate, tiny tie-flip risk)
MINE_DT = BF16


def ap3(t_, dims, off=0):
    """Strided free-dim view of a tile: dims = [[stride, size], ...]."""
    return bass.AP(tensor=t_.tensor, offset=t_.offset + off,
                   ap=[t_.ap[0]] + dims)


def build_nc():
    nc = bacc.Bacc("TRN2", target_bir_lowering=False, debug=False,
                   num_devices=NCORES)

    xbf_d = nc.dram_tensor("xbf", [B, C], BF16, kind="ExternalInput")
    xloc_d = nc.dram_tensor("xloc", [BL, C], F32, kind="ExternalInput")
    eqB_d = nc.dram_tensor("eqBh", [CT * 128, BL], BF16, kind="ExternalInput")
    cidb_d = nc.dram_tensor("cidb", [128, C], F32, kind="ExternalInput")
    ident_d = nc.dram_tensor("ident", [128, 128], F32, kind="ExternalInput")
    tcols_d = nc.dram_tensor("tcols", [128, NT], F32, kind="ExternalInput")
    gidxcol_d = nc.dram_tensor("gidxcol", [128, NT], F32, kind="ExternalInput")
    negjb_d = nc.dram_tensor("negjb", [128, BL], F32, kind="ExternalInput")
    bigoff_d = nc.dram_tensor("bigoff", [128, 1], F32, kind="ExternalInput")
    out_d = nc.dram_tensor("partial", [1, 1], F32, kind="ExternalOutput")

    # collective payload, per class ct*128+p:
    #   col ct        = vmax   (max over local rows of -hardness)
    #   col CT+ct     = negenc (BIGI - global row of local argmax, 0-masked)
    #   col 2CT+2ct+k = posenc (k=0 first, k=1 second local row enc)
    cc_in = nc.dram_tensor("cc_in", [128, 4 * CT], F32)
    cc_out = nc.dram_tensor("cc_out", [NCORES, 128, 4 * CT], F32)

    with tile.TileContext(nc) as tc:
        with (
            tc.tile_pool(name="persist", bufs=1) as pp,
            tc.tile_pool(name="scratch", bufs=2) as sp,
            tc.tile_pool(name="small", bufs=4) as smp,
            tc.tile_pool(name="gather", bufs=4) as gp,
            tc.tile_pool(name="psA", bufs=1, space="PSUM") as psa,
            tc.tile_pool(name="psR", bufs=1, space="PSUM") as psr,
        ):
            # ---------- tiny inputs + on-chip constants ----------
            tcols = pp.tile([128, NT], F32, tag="tcols")
            nc.sync.dma_start(out=tcols, in_=tcols_d.ap())
            gidxcol = pp.tile([128, NT], F32, tag="gidxcol")
            nc.sync.dma_start(out=gidxcol, in_=gidxcol_d.ap())
            negjb = pp.tile([128, BL], F32, tag="negjb")
            nc.sync.dma_start(out=negjb, in_=negjb_d.ap())
            bigoff = pp.tile([128, 1], F32, tag="bigoff")
            nc.sync.dma_start(out=bigoff, in_=bigoff_d.ap())

            cidbf = pp.tile([128, C], F32, tag="cidbf")
            nc.sync.dma_start(out=cidbf, in_=cidb_d.ap())
            identf = pp.tile([128, 128], F32, tag="identf")
            nc.sync.dma_start(out=identf, in_=ident_d.ap())
            ones = pp.tile([128, 1], F32, tag="ones")
            nc.vector.memset(ones, 1.0)
            shA = pp.tile([128, 1], F32, tag="shA")
            nc.vector.memset(shA, -SHIFT_A)
            shC = pp.tile([128, 1], F32, tag="shC")
            nc.vector.memset(shC, -SHIFT_C)

            # ---------- xloc loads (gate phase A) ----------
            xloc = []
            for t in range(NT):
                xt = pp.tile([128, C], F32, tag=f"xloc{t}")
                nc.sync.dma_start(out=xt, in_=xloc_d.ap()[t * 128:(t + 1) * 128, :])
                xloc.append(xt)

            ccall = pp.tile([128, 4 * CT], F32, tag="ccall")

            # ---------- phase B inputs: host-built one-hot eqB ----------
            eqB = []
            for ct in range(CT):
                eb = pp.tile([128, BL], BF16, tag=f"eqB{ct}")
                nc.sync.dma_start(out=eb, in_=eqB_d.ap()[ct * 128:(ct + 1) * 128, :])
                eqB.append(eb)

            # ---------- phase A: hardest-negative mining ----------
            dumps = []
            diags = []
            for t in range(NT):
                dump = pp.tile([128, C], MINE_DT, tag=f"dump{t}")
                rsum = smp.tile([128, 1], F32, tag="rsum")
                nc.scalar.activation(out=dump, in_=xloc[t], func=AF.Exp,
                                     bias=shA, scale=1.0, accum_out=rsum)
                rr = smp.tile([128, 1], F32, tag="rr")
                nc.vector.reciprocal(out=rr, in_=rsum)
                dg = pp.tile([128, 128], MINE_DT, tag=f"diag{t}")
                nc.vector.tensor_scalar(out=dg, in0=identf, scalar1=rr,
                                        scalar2=-1.0, op0=OP.mult, op1=OP.mult)
                dumps.append(dump)
                diags.append(dg)

            sav = pp.tile([128, 8 * CT], MINE_DT, tag="sav")
            sai = pp.tile([128, 8 * CT], U32, tag="sai")
            for g in range(4):
                psts = []
                for ci in range(2):
                    pst = psa.tile([128, C], F32, tag=f"pst{ci}")
                    psts.append(pst)
                    ct = g * 2 + ci
                    for t in range(NT):
                        nc.tensor.matmul(
                            pst[:, t * 128:(t + 1) * 128],
                            lhsT=dumps[t][:, ct * 128:(ct + 1) * 128],
                            rhs=diags[t], start=True, stop=True)
                for ci in range(2):
                    ct = g * 2 + ci
                    masked = sp.tile([128, C], MINE_DT, tag="masked")
                    nc.vector.scalar_tensor_tensor(
                        out=masked, in0=eqB[ct], scalar=-512.0,
                        in1=psts[ci], op0=OP.mult, op1=OP.add)
                    nc.vector.max(out=sav[:, ct * 8:(ct + 1) * 8], in_=masked)
                    nc.vector.max_index(out=sai[:, ct * 8:(ct + 1) * 8],
                                        in_max=sav[:, ct * 8:(ct + 1) * 8],
                                        in_values=masked)
            # wide encode
            nc.vector.tensor_copy(out=ccall[:, 0:CT], in_=ap3(sav, [[8, CT]]))
            aidx = smp.tile([128, CT], F32, tag="aidx")
            nc.vector.tensor_copy(out=aidx, in_=ap3(sai, [[8, CT]]))
            nc.vector.tensor_scalar(out=ccall[:, CT:2 * CT], in0=aidx,
                                    scalar1=bigoff, scalar2=-1.0,
                                    op0=OP.subtract, op1=OP.mult)

            # phase B: first/second local row per class via enc mult + MAX8
            sbv = pp.tile([128, 8 * CT], F32, tag="sbv")
            for ct in range(CT):
                enb = sp.tile([128, BL], F32, tag="enb")
                nc.gpsimd.tensor_tensor(out=enb, in0=eqB[ct], in1=negjb,
                                        op=OP.mult)
                nc.vector.max(out=sbv[:, ct * 8:(ct + 1) * 8], in_=enb)
            nc.vector.tensor_copy(out=ap3(ccall, [[2, CT], [1, 2]], off=2 * CT),
                                  in_=ap3(sbv, [[8, CT], [1, 2]]))

            nc.sync.dma_start(out=cc_in.ap(), in_=ccall)

            # ---------- AllGather ----------
            nc.gpsimd.collective_compute(
                "AllGather", OP.bypass,
                replica_groups=[list(range(NCORES))],
                ins=[cc_in.ap().opt()], outs=[cc_out.ap().opt()])

            # bubble work: eqm onehots + bf16 xloc for phase C
            eqm = []
            xlb = []
            for t in range(NT):
                em = pp.tile([128, C], BF16, tag=f"eqm{t}")
                nc.vector.tensor_scalar(out=em, in0=cidbf,
                                        scalar1=tcols[:, t:t + 1],
                                        scalar2=None, op0=OP.is_equal)
                eqm.append(em)
                xb = pp.tile([128, C], BF16, tag=f"xlb{t}")
                nc.scalar.copy(out=xb, in_=xloc[t])
                xlb.append(xb)

            # g8[p, core, qct] <- cc_out[core, p, qct]  (qct innermost, contig)
            NC8 = NCORES
            g8 = pp.tile([128, NC8 * 4 * CT], F32, tag="g8")
            gsrc = bass.AP(tensor=cc_out.ap().tensor, offset=0,
                           ap=[[4 * CT, 128], [128 * 4 * CT, NC8], [1, 4 * CT]])
            nc.scalar.dma_start(out=g8, in_=gsrc)

            # ---------- wide combine ----------
            W = 4 * CT
            negdims = [[1, CT], [W, NC8]]      # (ct, core), core innermost
            idx3 = smp.tile([128, 3 * CT], F32, tag="idx3")
            # neg: winner core by vmax, tie -> max enc
            gv = smp.tile([128, CT], F32, tag="gv")
            nc.vector.tensor_reduce(out=gv, in_=ap3(g8, negdims),
                                    axis=AX.X, op=OP.max)
            mmt = smp.tile([128, CT * NC8], F32, tag="mmt")
            mmdims = [[NC8, CT], [1, NC8]]
            nc.vector.tensor_tensor(out=ap3(mmt, mmdims),
                                    in0=ap3(g8, negdims),
                                    in1=ap3(gv, [[1, CT], [0, NC8]]),
                                    op=OP.is_ge)
            cand = smp.tile([128, CT * NC8], F32, tag="cand")
            nc.vector.tensor_tensor(out=ap3(cand, mmdims),
                                    in0=ap3(mmt, mmdims),
                                    in1=ap3(g8, negdims, off=CT),
                                    op=OP.mult)
            genc = smp.tile([128, CT], F32, tag="genc")
            nc.vector.tensor_reduce(out=genc, in_=ap3(cand, mmdims),
                                    axis=AX.X, op=OP.max)
            nc.vector.tensor_scalar(out=idx3[:, CT:2 * CT], in0=genc,
                                    scalar1=-1.0, scalar2=BIGI,
                                    op0=OP.mult, op1=OP.add)

            # pos: top2 enc of 16 candidates (2 local x 8 cores) per class
            g1e = smp.tile([128, CT], F32, tag="g1e")
            g2e = smp.tile([128, CT], F32, tag="g2e")
            for ct in range(CT):
                catsl = ap3(g8, [[W, NC8], [1, 2]], off=2 * CT + 2 * ct)
                top8 = smp.tile([128, 8], F32, tag="topg")
                nc.vector.max(out=top8, in_=catsl)
                nc.vector.tensor_copy(out=g1e[:, ct:ct + 1], in_=top8[:, 0:1])
                nc.vector.tensor_copy(out=g2e[:, ct:ct + 1], in_=top8[:, 1:2])
            nc.vector.tensor_scalar(out=idx3[:, 0:CT], in0=g1e,
                                    scalar1=-1.0, scalar2=BIGI,
                                    op0=OP.mult, op1=OP.add)
            g2i = smp.tile([128, CT], F32, tag="g2i")
            nc.vector.tensor_scalar(out=g2i, in0=g2e, scalar1=-1.0,
                                    scalar2=BIGI, op0=OP.mult, op1=OP.add)
            m2 = smp.tile([128, CT], F32, tag="m2")
            nc.vector.tensor_scalar(out=m2, in0=g2i, scalar1=BIGI,
                                    scalar2=None, op0=OP.is_lt)
            nc.vector.tensor_tensor(out=idx3[:, 2 * CT:3 * CT], in0=g2i,
                                    in1=m2, op=OP.mult)

            # ---------- bf16 hi/lo table ----------
            # idx = hi*128 + lo; tab cols ct*6+q hold hi*128, ct*6+3+q hold lo
            hsc = smp.tile([128, 3 * CT], F32, tag="hsc")
            nc.vector.tensor_scalar(out=hsc, in0=idx3, scalar1=1.0 / 128.0,
                                    scalar2=None, op0=OP.mult)
            hii = smp.tile([128, 3 * CT], I32, tag="hii")
            nc.vector.tensor_copy(out=hii, in_=hsc)  # trunc toward zero
            hif = smp.tile([128, 3 * CT], F32, tag="hif")
            nc.vector.tensor_copy(out=hif, in_=hii)
            hi128 = smp.tile([128, 3 * CT], F32, tag="hi128")
            nc.vector.tensor_scalar(out=hi128, in0=hif, scalar1=128.0,
                                    scalar2=None, op0=OP.mult)
            lof = smp.tile([128, 3 * CT], F32, tag="lof")
            nc.vector.tensor_tensor(out=lof, in0=idx3, in1=hi128, op=OP.subtract)
            tab = pp.tile([128, 6 * CT], BF16, tag="tab")
            nc.vector.tensor_copy(out=ap3(tab, [[1, 3], [6, CT]]),
                                  in_=ap3(hi128, [[CT, 3], [1, CT]]))
            nc.vector.tensor_copy(out=ap3(tab, [[1, 3], [6, CT]], off=3),
                                  in_=ap3(lof, [[CT, 3], [1, CT]]))

            # ---------- resolution ----------
            # ps6[q, j] = hi128_q[class(j)] (rows 0-2) / lo_q[class(j)] (3-5)
            ps6 = psr.tile([6, BL], F32, tag="ps6")
            for h in range(2):
                cols = slice(h * 512, (h + 1) * 512)
                for ct in range(CT):
                    nc.tensor.matmul(ps6[:, cols],
                                     lhsT=tab[:, ct * 6:(ct + 1) * 6],
                                     rhs=eqB[ct][:, cols],
                                     start=(ct == 0), stop=(ct == CT - 1))
            sb6 = pp.tile([6, BL], F32, tag="sb6")
            nc.scalar.copy(out=sb6, in_=ps6)

            # transpose per tile -> offp6[anchor, t*6 + {hi_q | lo_q}]
            offp6 = psr.tile([128, 6 * NT], F32, tag="offp6")
            for t in range(NT):
                nc.tensor.matmul(offp6[:, t * 6:(t + 1) * 6],
                                 lhsT=sb6[:, t * 128:(t + 1) * 128],
                                 rhs=identf[0:6, 0:6], start=True, stop=True)
            offs6 = smp.tile([128, 6 * NT], F32, tag="offs6")
            nc.scalar.copy(out=offs6, in_=offp6)
            # idxw[:, t*3+q] = hi*128 + lo
            idxw = smp.tile([128, 3 * NT], F32, tag="idxw")
            tq = [[3, NT], [1, 3]]
            nc.vector.tensor_tensor(out=ap3(idxw, tq),
                                    in0=ap3(offs6, [[6, NT], [1, 3]]),
                                    in1=ap3(offs6, [[6, NT], [1, 3]], off=3),
                                    op=OP.add)
            # pos = (g1 == self) ? p2z : g1   (wide, strided; q: 0=g1 1=neg 2=p2)
            m1 = smp.tile([128, NT], F32, tag="m1")
            nc.vector.tensor_tensor(out=m1, in0=ap3(idxw, [[3, NT]]),
                                    in1=gidxcol, op=OP.is_equal)
            dsel = smp.tile([128, NT], F32, tag="dsel")
            nc.vector.tensor_tensor(out=dsel, in0=ap3(idxw, [[3, NT]], off=2),
                                    in1=ap3(idxw, [[3, NT]]), op=OP.subtract)
            nc.vector.tensor_tensor(out=dsel, in0=dsel, in1=m1, op=OP.mult)
            posf = smp.tile([128, NT], F32, tag="posf")
            nc.vector.tensor_tensor(out=posf, in0=ap3(idxw, [[3, NT]]),
                                    in1=dsel, op=OP.add)
            offi = pp.tile([128, 2 * NT], I32, tag="offi")
            nc.vector.tensor_copy(out=ap3(offi, [[2, NT]]), in_=posf)
            nc.vector.tensor_copy(out=ap3(offi, [[2, NT]], off=1),
                                  in_=ap3(idxw, [[3, NT]], off=1))

            # ---------- gathers + phase C ----------
            lnstage = pp.tile([128, NT], F32, tag="lnstage")
            tvstage = pp.tile([128, NT], F32, tag="tvstage")
            exps3 = []
            for t in range(NT):
                gpair = gp.tile([128, 2 * C], BF16, tag="gpair")
                nc.gpsimd.indirect_dma_start(
                    out=gpair[:, 0:C], out_offset=None, in_=xbf_d.ap(),
                    in_offset=bass.IndirectOffsetOnAxis(
                        ap=offi[:, 2 * t:2 * t + 1], axis=0))
                nc.gpsimd.indirect_dma_start(
                    out=gpair[:, C:2 * C], out_offset=None, in_=xbf_d.ap(),
                    in_offset=bass.IndirectOffsetOnAxis(
                        ap=offi[:, 2 * t + 1:2 * t + 2], axis=0))
                sumpn = sp.tile([128, C], BF16, tag="sumpn")
                nc.vector.tensor_tensor(out=sumpn, in0=gpair[:, 0:C],
                                        in1=gpair[:, C:2 * C], op=OP.add)
                sum3 = sp.tile([128, C], BF16, tag="sum3")
                nc.vector.tensor_tensor(out=sum3, in0=sumpn, in1=xlb[t],
                                        op=OP.add)
                dumpc = sp.tile([128, C], BF16, tag="dumpc")
                e = nc.scalar.activation(out=dumpc, in_=sum3, func=AF.Exp,
                                         bias=shC, scale=1.0,
                                         accum_out=lnstage[:, t:t + 1])
                exps3.append(e)
                junk = sp.tile([128, C], BF16, tag="junk")
                nc.vector.tensor_tensor(out=junk, in0=sum3, in1=eqm[t],
                                        op=OP.mult)
                nc.vector.tensor_reduce(out=tvstage[:, t:t + 1], in_=junk,
                                        axis=AX.X, op=OP.add)

            lns = pp.tile([128, NT], F32, tag="lns")
            ln = nc.scalar.activation(out=lns, in_=lnstage, func=AF.Ln)
            tile.add_dep_helper(ln.ins, exps3[-1].ins, sync=False)
            li8 = smp.tile([128, NT], F32, tag="li8")
            nc.vector.tensor_scalar(out=li8, in0=lns, scalar1=SHIFT_C,
                                    scalar2=None, op0=OP.add)
            nc.vector.tensor_tensor(out=li8, in0=li8, in1=tvstage,
                                    op=OP.subtract)
            acc = smp.tile([128, 1], F32, tag="acc")
            nc.vector.tensor_reduce(out=acc, in_=li8, axis=AX.X, op=OP.add)

            pss = psr.tile([1, 1], F32, tag="psout")
            nc.tensor.matmul(pss, lhsT=acc, rhs=ones, start=True, stop=True)
            outt = smp.tile([1, 1], F32, tag="outt")
            nc.vector.tensor_copy(out=outt, in_=pss)
            nc.sync.dma_start(out=out_d.ap(), in_=outt)

    nc.compile()
    return nc


_NC_CACHE = {}


def get_nc():
    if "nc" not in _NC_CACHE:
        _NC_CACHE["nc"] = build_nc()
    return _NC_CACHE["nc"]


def make_in_maps(x, target):
    x = np.ascontiguousarray(np.asarray(x, dtype=np.float32))
    tgt = np.asarray(target).astype(np.int64)
    assert x.shape == (B, C) and tgt.shape == (B,)

    xbf = np.ascontiguousarray(x.astype(ml_dtypes.bfloat16))
    cid = np.arange(C, dtype=np.float32)
    cidb_full = np.ascontiguousarray(np.broadcast_to(cid, (128, C)))
    ident_full = np.eye(128, dtype=np.float32)


    in_maps = []
    for k in range(NCORES):
        rows = slice(k * BL, (k + 1) * BL)
        tl = tgt[rows].astype(np.float32)
        gi = (k * BL + np.arange(BL)).astype(np.float32)
        in_maps.append({
            "xbf": xbf,
            "xloc": np.ascontiguousarray(x[rows]),
            "cidb": cidb_full,
            "ident": ident_full,
            "eqBh": np.ascontiguousarray(
                (tgt[rows][None, :] == np.arange(C)[:, None])
                .astype(ml_dtypes.bfloat16)),
            "tcols": np.ascontiguousarray(tl.reshape(NT, 128).T),
            "gidxcol": np.ascontiguousarray(gi.reshape(NT, 128).T),
            "negjb": np.ascontiguousarray(np.broadcast_to(BIGI - gi, (128, BL))),
            "bigoff": np.full((128, 1), BIGI - k * BL, dtype=np.float32),
        })
    return in_maps


def kernel(x, target):
    nc = get_nc()
    in_maps = make_in_maps(x, target)
    res = run_bass_kernel_spmd(nc, in_maps, core_ids=list(range(NCORES)))
    total = sum(float(res.results[k]["partial"][0, 0]) for k in range(NCORES))
    return np.float32(total / B)
